# revision 14
# baseline (speedup 1.0000x reference)
"""DiscreteBipartiteFlow forward on 8 trn2 NeuronCores.

Math: inputs rows are exact one-hots (x0|x1). net = relu(x0@W1+b1)@W2+b2
only depends on i0=argmax(x0), so precompute (on device, per core) the
[V, 2V] table NET = relu(W1+b1)@W2+b2 and its per-row argmaxes
L[i]=argmax(NET[i,:V]), S[i]=argmax(NET[i,V:]). The straight-through
one_hot_argmax is numerically exactly-hard (off-argmax entries cancel to
exactly 0.0 in fp32), one_hot_multiply of a one-hot x1 by the one-hot
scale is an index product, and one_hot_add is an index sum, so
z1 = one_hot((L[i0] + a1*S[i0]) mod V) (or 0 when S[i0]==0, since scale
index 0 is excluded). Output = [x0 | z1].

Table lookups are packed base-128 into one broadcast row
Wpack[i] = L[i] + 128*S[i] + 16384*[S[i]>0] (exact small ints in fp32),
so the per-row work is two dot-products (tensor_mul + reduce_sum) on
DVE, a batched int32 unpack (power-of-2 mod via &/>>; the ALU `mod` op
is sim-only and rejected by walrus), and one compare-vs-iota — no PE in
the loop.
Data-parallel over 8 cores.

NOTE: this walrus build allows only ONE sync-wait command per compute
instruction (PE/DVE). The structure keeps every compute op at <=1 fresh
cross-engine dependency: tiny DVE "toucher" copies pre-cover DMA queues,
the identity dependency is absorbed once by a dummy transpose, and all
matmul operands are funneled through DVE.
"""

import numpy as np

V = 128
H = 512
N_CORES = 8
P = 128


def build_bass(rows: int):
    """Build the single-core Bass program for a [rows, 2V] batch shard."""
    import concourse.bacc as bacc
    import concourse.bass as bass  # noqa: F401
    import concourse.tile as tile
    from concourse import mybir

    f32 = mybir.dt.float32
    i32 = mybir.dt.int32
    A = mybir.AluOpType

    assert rows % P == 0
    nt = rows // P

    nc = bacc.Bacc(None)
    x = nc.declare_dram_parameter("x", [rows, 2 * V], f32, isOutput=False)
    w1 = nc.declare_dram_parameter("w1", [V, H], f32, isOutput=False)
    b1 = nc.declare_dram_parameter("b1", [P, H // P], f32, isOutput=False)
    w2 = nc.declare_dram_parameter("w2", [H, 2 * V], f32, isOutput=False)
    b2 = nc.declare_dram_parameter("b2", [1, 2 * V], f32, isOutput=False)
    out = nc.declare_dram_parameter("out", [rows, 2 * V], f32, isOutput=True)

    kh = H // P  # 4 chunks over the hidden dim

    with tile.TileContext(nc) as tc:
        with (
            tc.tile_pool(name="consts", bufs=1) as consts,
            tc.tile_pool(name="table", bufs=1) as table,
            tc.tile_pool(name="loop", bufs=2) as loop,
            tc.tile_pool(name="psum_t", bufs=2, space="PSUM") as psum_t,
            tc.tile_pool(name="psum_net", bufs=1, space="PSUM") as psum_net,
        ):
            # ---- constants (final writers on DVE) ----
            iota_i = consts.tile([P, P], i32)
            nc.gpsimd.iota(iota_i, pattern=[[1, P]], base=0, channel_multiplier=0)
            iota_f = consts.tile([P, P], f32)
            nc.vector.tensor_copy(iota_f, iota_i)
            # revio[c] = P - c: free-dim reduce_max of eq*revio picks the
            # smallest index among ties (matches jnp.argmax)
            revio = consts.tile([P, P], f32)
            nc.vector.tensor_scalar(
                out=revio, in0=iota_f, scalar1=-1.0, scalar2=float(P),
                op0=A.mult, op1=A.add,
            )
            ipart_i = consts.tile([P, 1], i32)
            nc.gpsimd.iota(ipart_i, pattern=[[1, 1]], base=0, channel_multiplier=1)
            ipart_f = consts.tile([P, 1], f32)
            nc.vector.tensor_copy(ipart_f, ipart_i)
            ones_row = consts.tile([1, P], f32)
            nc.vector.memset(ones_row, 1.0)
            # identity built on DVE: ident[p, c] = (c == p)
            ident = consts.tile([P, P], f32)
            nc.vector.tensor_scalar(
                out=ident, in0=iota_f, scalar1=ipart_f, scalar2=None, op0=A.is_equal,
            )
            # dummy transpose: absorbs the ident dependency into PE's clock
            scratch_ps = psum_t.tile([P, P], f32, tag="tp", bufs=2)
            nc.tensor.transpose(scratch_ps, ident, ident)

            # ---- load weights; DVE touchers pre-cover each DMA queue ----
            w1_sb = table.tile([P, H], f32)
            nc.sync.dma_start(out=w1_sb, in_=w1[:, :])
            w2_dma = table.tile([P, kh, 2 * V], f32)
            nc.sync.dma_start(out=w2_dma, in_=w2.rearrange("(k p) n -> p k n", p=P))
            b1_dma = table.tile([P, kh], f32)
            nc.sync.dma_start(out=b1_dma, in_=b1[:, :])
            b2_dma = table.tile([1, 2 * V], f32)
            nc.sync.dma_start(out=b2_dma, in_=b2[:, :])

            w1_touch = table.tile([P, 1], f32)
            nc.vector.tensor_copy(w1_touch, w1_sb[:, 0:1])
            w2_sb = table.tile([P, kh, 2 * V], f32)
            nc.vector.tensor_copy(w2_sb, w2_dma)
            b1_sb = table.tile([P, kh], f32)
            nc.vector.tensor_copy(b1_sb, b1_dma)
            b2_sb = table.tile([1, 2 * V], f32)
            nc.vector.tensor_copy(b2_sb, b2_dma)
            # joiner: one explicit DVE wait covering all weight-copy ticks so
            # later DVE readers of b1_sb/w2_sb/b2_sb carry no fresh wait
            join0 = table.tile([1, 1], f32)
            nc.vector.tensor_copy(join0, b2_sb[0:1, 0:1])
            join0b = table.tile([P, 1], f32)
            nc.vector.tensor_copy(join0b, b1_sb[:, 0:1])

            # ---- MLP table: NET = relu(W1 + b1) @ W2 + b2, [V, 2V] ----
            hT = table.tile([P, kh, P], f32)  # relu(W1+b1)^T, H on partitions
            for k in range(kh):
                w1t_ps = psum_t.tile([P, P], f32, tag="tp", bufs=2)
                nc.tensor.transpose(w1t_ps, w1_sb[:, k * P : (k + 1) * P], ident)
                # relu(w1t + b1) on DVE: (x + b1) max 0
                nc.vector.tensor_scalar(
                    out=hT[:, k, :], in0=w1t_ps, scalar1=b1_sb[:, k : k + 1],
                    scalar2=0.0, op0=A.add, op1=A.max,
                )
            net_ps = psum_net.tile([P, 2 * V], f32)
            for k in range(kh):
                nc.tensor.matmul(
                    net_ps, lhsT=hT[:, k, :], rhs=w2_sb[:, k, :],
                    start=(k == 0), stop=False,
                )
            nc.tensor.matmul(net_ps, lhsT=ones_row, rhs=b2_sb, start=False, stop=True)
            net_sb = table.tile([P, 2 * V], f32)
            nc.vector.tensor_copy(net_sb, net_ps)

            # ---- per-i0 argmax tables -> packed lookup row ----
            lsv = table.tile([P, 4], f32)  # cols: L, S, zflag, pack
            for head, col in ((0, 0), (1, 1)):
                seg = net_sb[:, head * V : (head + 1) * V]
                m = table.tile([P, 1], f32, tag=f"m{head}")
                nc.vector.reduce_max(m, seg, axis=mybir.AxisListType.X)
                eq = table.tile([P, P], f32, tag=f"eq{head}")
                nc.vector.tensor_scalar(out=eq, in0=seg, scalar1=m, scalar2=None, op0=A.is_equal)
                nc.vector.tensor_mul(eq, eq, revio)
                r = table.tile([P, 1], f32, tag=f"r{head}")
                nc.vector.reduce_max(r, eq, axis=mybir.AxisListType.X)
                nc.vector.tensor_scalar(
                    out=lsv[:, col : col + 1], in0=r, scalar1=-1.0, scalar2=float(P),
                    op0=A.mult, op1=A.add,
                )
            # zflag = [S > 0] (scale index 0 is excluded in one_hot_multiply)
            nc.vector.tensor_scalar(
                out=lsv[:, 2:3], in0=lsv[:, 1:2], scalar1=0.5, scalar2=None, op0=A.is_gt,
            )
            # pack = L + 128*S + 16384*zflag  (exact ints < 2^24)
            nc.vector.tensor_scalar(
                out=lsv[:, 3:4], in0=lsv[:, 1:2], scalar1=float(V),
                scalar2=lsv[:, 0:1], op0=A.mult, op1=A.add,
            )
            nc.vector.tensor_scalar(
                out=lsv[:, 3:4], in0=lsv[:, 2:3], scalar1=float(V * V),
                scalar2=lsv[:, 3:4], op0=A.mult, op1=A.add,
            )
            # transpose pack [P,1] -> [1,P] via matmul, then broadcast to all
            # partitions via a K=1 outer product with ones
            packT_ps = psum_t.tile([1, P], f32, tag="pk", bufs=1)
            nc.tensor.matmul(packT_ps, lhsT=lsv[:, 3:4], rhs=ident, start=True, stop=True)
            packrow = table.tile([1, P], f32)
            nc.vector.tensor_copy(packrow, packT_ps)
            wpack_ps = psum_t.tile([P, P], f32, tag="tp", bufs=2)
            nc.tensor.matmul(wpack_ps, lhsT=ones_row, rhs=packrow, start=True, stop=True)
            wpack = table.tile([P, P], f32)
            nc.vector.tensor_copy(wpack, wpack_ps)
            join1 = table.tile([P, 1], f32)
            nc.vector.tensor_copy(join1, wpack[:, 0:1])

            # ---- main loop over row tiles: DVE + DMA only ----
            # Phase A: per tile, two dot-products (one-hot selects) into
            # packed columns of W_all/A_all.
            w_all = table.tile([P, nt], f32)   # packed table value per row
            a_all = table.tile([P, nt], f32)   # a1 index per row
            xts = []
            for t in range(nt):
                rs = t * P
                xt = loop.tile([P, 2 * V], f32, tag="xt", bufs=nt)
                nc.sync.dma_start(out=xt, in_=x[rs : rs + P, :])
                xts.append(xt)
                sc0 = loop.tile([P, V], f32, tag="sc0", bufs=2)
                nc.vector.tensor_mul(sc0, xt[:, 0:V], wpack)
                nc.vector.reduce_sum(w_all[:, t : t + 1], sc0, axis=mybir.AxisListType.X)
                sc1 = loop.tile([P, V], f32, tag="sc1", bufs=2)
                nc.vector.tensor_mul(sc1, xt[:, V : 2 * V], iota_f)
                nc.vector.reduce_sum(a_all[:, t : t + 1], sc1, axis=mybir.AxisListType.X)
                # pass-through half can stream out immediately
                nc.sync.dma_start(out=out[rs : rs + P, 0:V], in_=xt[:, 0:V])

            # Phase B: batched unpack across all tiles (wide [P, nt] ops).
            # w = L + 128*S + 16384*zflag; a1 as loaded.
            # zm = [w >= 16384]; rem = w - 16384*zm; l = rem & 127;
            # s = rem >> 7; t = s*a1 + l; c = t & 127   (ints exact)
            zm = table.tile([P, nt], f32)
            nc.vector.tensor_scalar(
                out=zm, in0=w_all, scalar1=float(V * V), scalar2=None, op0=A.is_ge,
            )
            tmp = table.tile([P, nt], f32)
            nc.vector.tensor_scalar(
                out=tmp, in0=zm, scalar1=float(V * V), scalar2=None, op0=A.mult,
            )
            rem = table.tile([P, nt], f32)
            nc.vector.tensor_sub(rem, w_all, tmp)
            remi = table.tile([P, nt], i32)
            nc.vector.tensor_copy(remi, rem)
            a1i = table.tile([P, nt], i32)
            nc.vector.tensor_copy(a1i, a_all)
            li = table.tile([P, nt], i32)
            nc.vector.tensor_scalar(
                out=li, in0=remi, scalar1=V - 1, scalar2=None, op0=A.bitwise_and,
            )
            si = table.tile([P, nt], i32)
            nc.vector.tensor_scalar(
                out=si, in0=remi, scalar1=7, scalar2=None, op0=A.arith_shift_right,
            )
            ti = table.tile([P, nt], i32)
            nc.vector.tensor_mul(ti, si, a1i)
            nc.vector.tensor_add(ti, ti, li)
            ci = table.tile([P, nt], i32)
            nc.vector.tensor_scalar(
                out=ci, in0=ti, scalar1=V - 1, scalar2=None, op0=A.bitwise_and,
            )
            cf = table.tile([P, nt], f32)
            nc.vector.tensor_copy(cf, ci)

            # Phase C: per tile, one-hot build + store.
            for t in range(nt):
                rs = t * P
                zt = loop.tile([P, V], f32, tag="zt", bufs=nt)
                nc.vector.tensor_scalar(
                    out=zt, in0=iota_f, scalar1=cf[:, t : t + 1],
                    scalar2=zm[:, t : t + 1], op0=A.is_equal, op1=A.mult,
                )
                nc.sync.dma_start(out=out[rs : rs + P, V : 2 * V], in_=zt)

    # Bacc.finalize -> compile(): move_matmul_waits_to_ldweights +
    # generate_event_semaphores split multi-wait instructions down to the
    # TRN2 1-wait-per-instruction hardware limit.
    nc.finalize()
    return nc


# Test-harness hooks: extra kwargs for run_bass_kernel_spmd (e.g. trace=True)
# and the last BassKernelResults for profiling. Unused when graded.
RUN_KWARGS: dict = {}
LAST_RESULTS = None


def kernel(**inputs) -> np.ndarray:
    global LAST_RESULTS
    from concourse.bass_utils import run_bass_kernel_spmd

    x = np.ascontiguousarray(np.asarray(inputs["inputs"], dtype=np.float32))
    W1 = np.ascontiguousarray(np.asarray(inputs["W1"], dtype=np.float32))
    b1 = np.ascontiguousarray(
        np.asarray(inputs["b1"], dtype=np.float32).reshape(H // P, P).T
    )  # [P, kh]: partition p of chunk k holds b1[k*P + p]
    W2 = np.ascontiguousarray(np.asarray(inputs["W2"], dtype=np.float32))
    b2 = np.ascontiguousarray(np.asarray(inputs["b2"], dtype=np.float32).reshape(1, 2 * V))

    B = x.shape[0]
    rows = B // N_CORES
    nc = build_bass(rows)

    shards = np.split(x, N_CORES, axis=0)
    in_maps = [
        {"x": s, "w1": W1, "b1": b1, "w2": W2, "b2": b2} for s in shards
    ]
    res = run_bass_kernel_spmd(nc, in_maps, list(range(N_CORES)), **RUN_KWARGS)
    LAST_RESULTS = res
    return np.concatenate([r["out"] for r in res.results], axis=0)


if __name__ == "__main__":
    rng = np.random.default_rng(0)
    idx = rng.integers(0, V, size=(256, 2))
    x = np.zeros((256, 2 * V), np.float32)
    x[np.arange(256), idx[:, 0]] = 1
    x[np.arange(256), V + idx[:, 1]] = 1
    ins = {
        "inputs": x,
        "W1": rng.standard_normal((V, H), dtype=np.float32) / np.sqrt(V),
        "b1": np.zeros(H, np.float32),
        "W2": rng.standard_normal((H, 2 * V), dtype=np.float32) / np.sqrt(H),
        "b2": np.zeros(2 * V, np.float32),
    }
    print(kernel(**ins).shape)
